# revision 1
# baseline (speedup 1.0000x reference)
"""TRN2 Bass kernel for nn_EqLayer (GNN message passing), 8 NeuronCores.

Strategy (edge/graph parallelism): nodes are split into 8 contiguous dst
ranges; each core owns the edges whose dst falls in its range and produces
that range's output rows (host only concatenates — no cross-core reduction).
Per core, edges are grouped by 128-node dst window and by src-id half (so
gather indices fit int16), each (window, half) block padded to a multiple of
128 with inert edges.

Device pipeline per window:
- dma_gather (transpose mode) pulls 512B node tokens feature-on-partition.
  Token = [rot-expanded 128 | scalars 32 | pad]: rot features are duplicated
  over the rotation output index l (q = j*16+k*4+m*2+l), which turns the
  per-edge 2x2 rotation into ONE elementwise multiply with a host-built
  per-edge table; the sum over m rides the matmul contraction with W1 rows
  replicated (K 64->128).
- MLP1 = 5 PSUM-accumulated matmuls; silu+b1 on ScalarE during PSUM->SBUF.
- MLP2 per 128-edge chunk with h as lhsT -> PSUM [128 edges, 160] where the
  160 columns are [scal 32 | rot-expanded 128] via W2 column duplication.
- Output rotation = one multiply with the edge-major table T_out.
- Segment-sum: one-hot [128e, 128n] built by iota/is_equal vs the chunk's
  local dst ids, then a matmul accumulates all of a window's chunks into the
  window's PSUM tile; m-pairs are folded on evacuation and the 128-node row
  block is written out contiguously.
"""

import sys

sys.path.insert(0, "/opt/trn_rl_repo")

import numpy as np
import ml_dtypes

import concourse.bass as bass
import concourse.bacc as bacc
import concourse.mybir as mybir
from concourse import tile
from concourse.bass_utils import run_bass_kernel_spmd

BF16 = mybir.dt.bfloat16
FP32 = mybir.dt.float32
I16 = mybir.dt.int16
BF = ml_dtypes.bfloat16

N_CORES = 8
N_SCALARS = 32
NUM_REP = 8
L_MAX = 4
HIDDEN = 128
DIST_DIM = 64
ROT_EXP = 128
MSG_COLS = N_SCALARS + ROT_EXP  # 160
TOK = 256  # bf16 elems per token (512B)
HALF = 32768
WIN = 128
CHUNK = 128
TILE_E = 512


def _wrap_idx16(vals):
    L = len(vals)
    w = np.asarray(vals, np.int64).reshape(L // 16, 16).T.astype(np.int16)
    return np.tile(w, (8, 1))


def _prep_edges(src_g, dst_g, n_nodes, half):
    n_local = n_nodes // N_CORES
    n_win = (n_local + WIN - 1) // WIN
    owner = np.minimum(dst_g // n_local, N_CORES - 1)
    sizes = []
    per_core_blocks = []
    for c in range(N_CORES):
        base = c * n_local
        sel = np.where(owner == c)[0]
        dloc = dst_g[sel] - base
        w_loc = dloc >> 7
        is_b = (src_g[sel] >= half).astype(np.int8)
        order = np.lexsort((is_b, w_loc))
        core_blocks = []
        for w in range(n_win):
            m = order[(w_loc[order] == w)]
            a = m[is_b[m] == 0]
            b = m[is_b[m] == 1]
            core_blocks.append((sel[a], sel[b]))
            sizes.append((len(a), len(b)))
        per_core_blocks.append(core_blocks)
    LA = max(128, ((max(s[0] for s in sizes) + 127) // 128) * 128)
    LB = max(128, ((max(s[1] for s in sizes) + 127) // 128) * 128)
    per_core = []
    for c in range(N_CORES):
        perm = np.full(n_win * (LA + LB), -1, np.int64)
        off = 0
        for w in range(n_win):
            a, b = per_core_blocks[c][w]
            perm[off : off + len(a)] = a
            perm[off + LA : off + LA + len(b)] = b
            off += LA + LB
        per_core.append({"perm": perm, "base": c * n_local})
    return LA, LB, n_win, n_local, per_core


def _build_tokens(x_scalar, x_rot):
    N = x_scalar.shape[0]
    tok = np.zeros((N, TOK), np.float32)
    xr = x_rot.reshape(N, NUM_REP * L_MAX * 2)
    tok[:, :ROT_EXP] = np.repeat(xr, 2, axis=1)
    tok[:, ROT_EXP : ROT_EXP + N_SCALARS] = x_scalar
    return tok.astype(BF)


def _expand_w1(W1):
    out = np.zeros((384, W1.shape[1]), np.float32)
    out[0:32] = W1[0:32]
    out[160:192] = W1[96:128]
    out[320:384] = W1[192:256]
    j, k, m, l = np.meshgrid(
        np.arange(8), np.arange(4), np.arange(2), np.arange(2), indexing="ij"
    )
    q = (j * 16 + k * 4 + m * 2 + l).ravel()
    row = (j * 8 + k * 2 + l).ravel()
    out[32 + q] = W1[32 + row]
    out[192 + q] = W1[128 + row]
    return out


def _expand_w2(W2):
    out = np.zeros((W2.shape[0], MSG_COLS), np.float32)
    out[:, :32] = W2[:, :32]
    j, k, m, l = np.meshgrid(
        np.arange(8), np.arange(4), np.arange(2), np.arange(2), indexing="ij"
    )
    q = (j * 16 + k * 4 + m * 2 + l).ravel()
    col = (j * 8 + k * 2 + m).ravel()
    out[:, 32 + q] = W2[:, 32 + col]
    return out


def _build_core_inputs(perm, base, n_local, half, src_g, dst_g, dist, rot, tok, W1p, b1, W2p):
    e_pad = len(perm)
    valid = perm >= 0
    pidx = np.where(valid, perm, 0)

    src = np.where(valid, src_g[pidx], 0)
    dst_loc = np.where(valid, dst_g[pidx] - base, 0)
    winloc = np.where(valid, dst_loc - ((dst_loc >> 7) << 7), -1).astype(np.float32)

    r = rot[pidx].reshape(e_pad, L_MAX, 2, 2).astype(np.float32)
    r = r * valid[:, None, None, None]
    kml_in = np.transpose(r, (0, 1, 3, 2)).reshape(e_pad, 16)
    kml_out = r.reshape(e_pad, 16)
    t_in = np.tile(kml_in, (1, NUM_REP)).T.copy()
    t_out = np.tile(kml_out, (1, NUM_REP))

    dist_e = dist[pidx].astype(np.float32) * valid[:, None]

    idx_a = np.where(valid & (src < half), src, 0)
    idx_b = np.where(valid & (src >= half), src - half, 0)
    idx_dst = np.where(valid, dst_loc, 0)

    return {
        "tok_g": np.ascontiguousarray(tok),
        "tok_l": np.ascontiguousarray(tok[base : base + n_local]),
        "t_in": t_in.astype(BF),
        "t_out": t_out.astype(BF),
        "dist_t": dist_e.T.copy().astype(BF),
        "winloc": winloc,
        "idx_src": _wrap_idx16(idx_a + idx_b),
        "idx_dst": _wrap_idx16(idx_dst),
        "w1p": W1p.astype(BF),
        "b1t": b1.reshape(HIDDEN, 1).astype(np.float32),
        "w2p": W2p.astype(BF),
    }


def _build_kernel(LA, LB, n_win, n_tok_global, n_local, half):
    e_pad = n_win * (LA + LB)
    out_rows = n_win * WIN

    nc = bacc.Bacc(
        "TRN2", target_bir_lowering=False, debug=False, num_devices=N_CORES
    )

    tok_g = nc.dram_tensor("tok_g", [n_tok_global, TOK], BF16, kind="ExternalInput")
    tok_l = nc.dram_tensor("tok_l", [n_local, TOK], BF16, kind="ExternalInput")
    t_in = nc.dram_tensor("t_in", [ROT_EXP, e_pad], BF16, kind="ExternalInput")
    t_out = nc.dram_tensor("t_out", [e_pad, ROT_EXP], BF16, kind="ExternalInput")
    dist_t = nc.dram_tensor("dist_t", [DIST_DIM, e_pad], BF16, kind="ExternalInput")
    winloc = nc.dram_tensor("winloc", [e_pad], FP32, kind="ExternalInput")
    idx_src = nc.dram_tensor("idx_src", [128, e_pad // 16], I16, kind="ExternalInput")
    idx_dst = nc.dram_tensor("idx_dst", [128, e_pad // 16], I16, kind="ExternalInput")
    w1p = nc.dram_tensor("w1p", [384, HIDDEN], BF16, kind="ExternalInput")
    b1t = nc.dram_tensor("b1t", [HIDDEN, 1], FP32, kind="ExternalInput")
    w2p = nc.dram_tensor("w2p", [HIDDEN, MSG_COLS], BF16, kind="ExternalInput")
    out_d = nc.dram_tensor("out", [out_rows, 96], FP32, kind="ExternalOutput")

    AF = mybir.ActivationFunctionType
    OP = mybir.AluOpType

    with tile.TileContext(nc) as tc:
        with (
            tc.tile_pool(name="const", bufs=1) as constp,
            tc.tile_pool(name="stream", bufs=2) as streamp,
            tc.tile_pool(name="work", bufs=3) as workp,
            tc.tile_pool(name="msgp", bufs=4) as msgp,
            tc.tile_pool(name="ph", bufs=2, space="PSUM") as ph_pool,
            tc.tile_pool(name="pm", bufs=2, space="PSUM") as pm_pool,
            tc.tile_pool(name="pw", bufs=2, space="PSUM") as pw_pool,
        ):
            w1_dscal = constp.tile([32, HIDDEN], BF16, tag="w1a")
            nc.sync.dma_start(w1_dscal[:], w1p[0:32, :])
            w1_drot = constp.tile([128, HIDDEN], BF16, tag="w1b")
            nc.sync.dma_start(w1_drot[:], w1p[32:160, :])
            w1_sscal = constp.tile([32, HIDDEN], BF16, tag="w1c")
            nc.sync.dma_start(w1_sscal[:], w1p[160:192, :])
            w1_srot = constp.tile([128, HIDDEN], BF16, tag="w1d")
            nc.sync.dma_start(w1_srot[:], w1p[192:320, :])
            w1_dist = constp.tile([64, HIDDEN], BF16, tag="w1e")
            nc.sync.dma_start(w1_dist[:], w1p[320:384, :])
            b1_t = constp.tile([HIDDEN, 1], FP32, tag="b1")
            nc.sync.dma_start(b1_t[:], b1t[:])
            w2_t = constp.tile([HIDDEN, MSG_COLS], BF16, tag="w2")
            nc.sync.dma_start(w2_t[:], w2p[:])
            iota_t = constp.tile([128, 128], FP32, tag="iota")
            nc.gpsimd.iota(
                iota_t[:], pattern=[[1, 128]], base=0, channel_multiplier=0,
                allow_small_or_imprecise_dtypes=True,
            )
            idxs_t = constp.tile([128, e_pad // 16], I16, tag="idxs")
            nc.sync.dma_start(idxs_t[:], idx_src[:])
            idxd_t = constp.tile([128, e_pad // 16], I16, tag="idxd")
            nc.sync.dma_start(idxd_t[:], idx_dst[:])

            mm = nc.tensor.matmul
            EC = LA + LB

            for w in range(n_win):
                estart = w * EC
                g_a = streamp.tile([128, 2, LA], BF16, tag="ga")
                g_b = streamp.tile([128, 2, LB], BF16, tag="gb")
                g_d = streamp.tile([128, 2, EC], BF16, tag="gd")
                tin_s = streamp.tile([ROT_EXP, EC], BF16, tag="tin")
                tout_s = streamp.tile([128, EC // 128, 128], BF16, tag="tout")
                dist_s = streamp.tile([DIST_DIM, EC], BF16, tag="dist")
                wl_s = streamp.tile([128, EC // 128], FP32, tag="wl")

                nc.sync.dma_start(tin_s[:], t_in[:, estart : estart + EC])
                nc.sync.dma_start(
                    tout_s[:],
                    t_out.ap()[estart : estart + EC, :].rearrange(
                        "(c p) q -> p c q", p=128
                    ),
                )
                nc.sync.dma_start(dist_s[:], dist_t[:, estart : estart + EC])
                nc.sync.dma_start(
                    wl_s[:],
                    winloc.ap()[estart : estart + EC].rearrange("(c p) -> p c", p=128),
                )
                nc.gpsimd.dma_gather(
                    g_a[:], tok_g.ap(),
                    idxs_t[:, estart // 16 : (estart + LA) // 16],
                    LA, LA, TOK, transpose=True, single_packet=False,
                )
                nc.gpsimd.dma_gather(
                    g_b[:], tok_g.ap()[half:, :],
                    idxs_t[:, (estart + LA) // 16 : (estart + EC) // 16],
                    LB, LB, TOK, transpose=True, single_packet=False,
                )
                nc.gpsimd.dma_gather(
                    g_d[:], tok_l.ap(),
                    idxd_t[:, estart // 16 : (estart + EC) // 16],
                    EC, EC, TOK, transpose=True, single_packet=False,
                )

                pw_t = pw_pool.tile([128, MSG_COLS], FP32, tag="pwin")
                nch = EC // CHUNK
                nch_done = 0

                spans = []
                for a0 in range(0, LA, TILE_E):
                    spans.append((a0, min(TILE_E, LA - a0), g_a, a0))
                for b0 in range(0, LB, TILE_E):
                    spans.append((LA + b0, min(TILE_E, LB - b0), g_b, b0))

                for (e0, te, g_t, go) in spans:
                    v_src = workp.tile([ROT_EXP, TILE_E], BF16, tag="vsrc")
                    v_dst = workp.tile([ROT_EXP, TILE_E], BF16, tag="vdst")
                    nc.vector.tensor_tensor(
                        v_src[:, :te], g_t[:, 0, go : go + te],
                        tin_s[:, e0 : e0 + te], OP.mult,
                    )
                    nc.vector.tensor_tensor(
                        v_dst[:, :te], g_d[:, 0, e0 : e0 + te],
                        tin_s[:, e0 : e0 + te], OP.mult,
                    )
                    psum_h = ph_pool.tile([128, TILE_E], FP32, tag="ph")
                    mm(psum_h[:, :te], w1_dscal[:], g_d[0:32, 1, e0 : e0 + te],
                       start=True, stop=False, skip_group_check=True)
                    mm(psum_h[:, :te], w1_drot[:], v_dst[:, :te],
                       start=False, stop=False, skip_group_check=True)
                    mm(psum_h[:, :te], w1_sscal[:], g_t[0:32, 1, go : go + te],
                       start=False, stop=False, skip_group_check=True)
                    mm(psum_h[:, :te], w1_srot[:], v_src[:, :te],
                       start=False, stop=False, skip_group_check=True)
                    mm(psum_h[:, :te], w1_dist[:], dist_s[:, e0 : e0 + te],
                       start=False, stop=True, skip_group_check=True)
                    h_t = workp.tile([128, TILE_E], BF16, tag="h")
                    nc.scalar.activation(
                        h_t[:, :te], psum_h[:, :te], AF.Silu, bias=b1_t[:], scale=1.0
                    )
                    for c in range(te // CHUNK):
                        ci = e0 // CHUNK + c
                        psum_m = pm_pool.tile([128, MSG_COLS], FP32, tag="pm")
                        mm(psum_m[:], h_t[:, c * 128 : (c + 1) * 128], w2_t[:],
                           start=True, stop=True, skip_group_check=True)
                        msg = msgp.tile([128, MSG_COLS], BF16, tag="msg")
                        nc.vector.tensor_tensor(
                            msg[:, N_SCALARS:], psum_m[:, N_SCALARS:],
                            tout_s[:, ci, :], OP.mult,
                        )
                        nc.vector.tensor_copy(msg[:, :N_SCALARS], psum_m[:, :N_SCALARS])
                        oh = msgp.tile([128, 128], BF16, tag="oh")
                        nc.vector.tensor_scalar(
                            oh[:], iota_t[:], wl_s[:, ci : ci + 1], None, OP.is_equal
                        )
                        nch_done += 1
                        mm(pw_t[:], oh[:], msg[:],
                           start=(nch_done == 1), stop=(nch_done == nch),
                           skip_group_check=True)

                acc = workp.tile([128, MSG_COLS], FP32, tag="acc")
                nc.vector.tensor_copy(acc[:], pw_t[:])
                folded = workp.tile([128, 96], FP32, tag="folded")
                nc.vector.tensor_copy(folded[:, :N_SCALARS], acc[:, :N_SCALARS])
                accr = acc[:, N_SCALARS:].rearrange(
                    "p (j k m l) -> p j k m l", j=8, k=4, m=2
                )
                fr = folded[:, N_SCALARS:].rearrange("p (j k l) -> p j k l", j=8, k=4)
                nc.vector.tensor_tensor(
                    fr, accr[:, :, :, 0, :], accr[:, :, :, 1, :], OP.add
                )
                nc.sync.dma_start(out_d.ap()[w * WIN : (w + 1) * WIN, :], folded[:])

    nc.compile()
    return nc


def kernel(x_scalar, x_rot, edge_index, distance_embedding, rot, W1, b1, W2, b2):
    x_scalar = np.asarray(x_scalar, np.float32)
    x_rot = np.asarray(x_rot, np.float32)
    edge_index = np.asarray(edge_index).astype(np.int64)
    distance_embedding = np.asarray(distance_embedding, np.float32)
    rot = np.asarray(rot, np.float32)
    W1 = np.asarray(W1, np.float32)
    b1 = np.asarray(b1, np.float32)
    W2 = np.asarray(W2, np.float32)
    b2 = np.asarray(b2, np.float32)

    N = x_scalar.shape[0]
    src, dst = edge_index[0], edge_index[1]

    LA, LB, n_win, n_local, per_core = _prep_edges(src, dst, N, HALF)
    tok = _build_tokens(x_scalar, x_rot)
    W1p = _expand_w1(W1)
    W2p = _expand_w2(W2)
    in_maps = [
        _build_core_inputs(
            pc["perm"], pc["base"], n_local, HALF,
            src, dst, distance_embedding, rot, tok, W1p, b1, W2p,
        )
        for pc in per_core
    ]
    nc = _build_kernel(LA, LB, n_win, N, n_local, HALF)
    res = run_bass_kernel_spmd(nc, in_maps, core_ids=list(range(N_CORES)))

    out_scalar = np.zeros((N, N_SCALARS), np.float32)
    out_rot = np.zeros((N, NUM_REP, L_MAX * 2), np.float32)
    for c in range(N_CORES):
        o = res.results[c]["out"][:n_local]
        out_scalar[c * n_local : (c + 1) * n_local] = o[:, :N_SCALARS]
        out_rot[c * n_local : (c + 1) * n_local] = o[:, N_SCALARS:].reshape(
            n_local, NUM_REP, L_MAX * 2
        )

    # b2 is applied per edge before the output rotation + segment sum; fold it
    # in on the host (zero in this benchmark, so this is a no-op).
    if np.any(b2):
        deg = np.bincount(dst, minlength=N).astype(np.float32)
        out_scalar += deg[:, None] * b2[:N_SCALARS]
        rsum = np.zeros((N, L_MAX, 2, 2), np.float32)
        np.add.at(rsum, dst, rot)
        b2r = b2[N_SCALARS:].reshape(NUM_REP, L_MAX, 2)
        corr = np.einsum("jkm,nkml->njkl", b2r, rsum)
        out_rot += corr.reshape(N, NUM_REP, L_MAX * 2)

    return out_scalar, out_rot
